# revision 20
# baseline (speedup 1.0000x reference)
"""Trainium2 Bass kernel for nn_DualEncoderGraphTransformer.

8-core SPMD: nodes sharded by range (6250/core, padded to 6272 = 49 blocks
x 128), edges bucketed by dst owner -> dst block -> src lo/hi half (int16
gather index limit). Per-edge attention via dma_gather of kv rows + one-hot
S^T matmul segment softmax (no segment-max needed: alpha in [-0.15, 0.15]).
Node features exchanged between stages with AllGather; pooled output combined
with AllReduce.

Self-contained: hardcodes all shapes for the fixed problem size.
"""
import numpy as np
import ml_dtypes

import concourse.bass as bass
import concourse.bacc as bacc
import concourse.mybir as mybir
import concourse.tile as tile
from concourse.bass_utils import run_bass_kernel_spmd
from concourse.masks import make_identity

NCORES = 8
N, E, IN_C, HID, HEADS, OUT_C, NG = 50000, 800000, 300, 64, 2, 2, 256
NPC = N // NCORES  # 6250 nodes per core
NBLK = 49
NLOC = NBLK * 128  # 6272 padded local nodes
NPAD = NCORES * NLOC  # 50176 padded global ids
SPLIT = 32768
KPAD = 384  # IN_C padded to 3x128

bf16 = mybir.dt.bfloat16
f32 = mybir.dt.float32
i16 = mybir.dt.int16
nbf = ml_dtypes.bfloat16


def _wrap_idx16(vals, cap):
    """dma_gather idx layout: [128, cap/16] int16; element i at partition
    i%16, col i//16; replicated 8x across partition groups."""
    arr = np.zeros(cap, np.int16)
    arr[: len(vals)] = vals
    a16 = arr.reshape(cap // 16, 16).T
    return np.tile(a16, (8, 1))


def _prep(inputs):
    x = np.asarray(inputs["x"], np.float32)
    ei = np.asarray(inputs["edge_index"]).astype(np.int64)
    batch = np.asarray(inputs["batch"]).astype(np.int64)
    src, dst = ei[0], ei[1]

    owner = np.minimum(dst // NPC, NCORES - 1)
    dstloc = dst - owner * NPC  # 0..6249
    blk = dstloc // 128
    src_pad = (src // NPC) * NLOC + (src % NPC)
    lo = src_pad < SPLIT

    # bucket edges by (owner, blk, ~lo) -- lo first
    order = np.lexsort((~lo, blk, owner))
    so, sb, slo = owner[order], blk[order], lo[order]
    s_srcpad, s_dstloc = src_pad[order], dstloc[order]

    # counts per (owner, blk)
    key = so * NBLK + sb
    nlo_counts = np.bincount(key[slo], minlength=NCORES * NBLK).reshape(NCORES, NBLK)
    nall_counts = np.bincount(key, minlength=NCORES * NBLK).reshape(NCORES, NBLK)
    nhi_counts = nall_counts - nlo_counts
    CAPL = int(-(-nlo_counts.max() // 128) * 128)
    CAPH = int(-(-nhi_counts.max() // 128) * 128)
    NTL, NTH = CAPL // 128, CAPH // 128
    NT = NTL + NTH

    # bucket start offsets in the sorted edge array
    bucket_starts = np.zeros(NCORES * NBLK + 1, np.int64)
    np.cumsum(nall_counts.ravel(), out=bucket_starts[1:])

    kv_ilo = np.zeros((NCORES, NBLK, 128, CAPL // 16), np.int16)
    kv_ihi = np.zeros((NCORES, NBLK, 128, CAPH // 16), np.int16)
    q_ilo = np.zeros((NCORES, NBLK, 128, CAPL // 16), np.int16)
    q_ihi = np.zeros((NCORES, NBLK, 128, CAPH // 16), np.int16)
    dstrel = np.full((NCORES, NBLK, 128, NT), -1.0, nbf)

    for c in range(NCORES):
        for b in range(NBLK):
            i0 = bucket_starts[c * NBLK + b]
            nl = int(nlo_counts[c, b])
            na = int(nall_counts[c, b])
            lo_sp = s_srcpad[i0 : i0 + nl]
            lo_dl = s_dstloc[i0 : i0 + nl]
            hi_sp = s_srcpad[i0 + nl : i0 + na] - SPLIT
            hi_dl = s_dstloc[i0 + nl : i0 + na]
            kv_ilo[c, b] = _wrap_idx16(lo_sp, CAPL)
            kv_ihi[c, b] = _wrap_idx16(hi_sp, CAPH)
            q_ilo[c, b] = _wrap_idx16(lo_dl, CAPL)
            q_ihi[c, b] = _wrap_idx16(hi_dl, CAPH)
            base = b * 128
            for i in range(nl):
                dstrel[c, b, i % 128, i // 128] = lo_dl[i] - base
            nh = na - nl
            for i in range(nh):
                dstrel[c, b, i % 128, NTL + i // 128] = hi_dl[i] - base

    # pooling prep
    cnt = np.bincount(batch, minlength=NG).astype(np.float32)
    g0 = np.array([batch[c * NPC] for c in range(NCORES)], np.int64)
    span = np.array([batch[(c + 1) * NPC - 1] - g0[c] for c in range(NCORES)])
    assert span.max() < 64, span.max()
    batchrel = np.full((NCORES, NBLK, 128, 1), -1.0, nbf)
    wnode = np.zeros((NCORES, NBLK, 128, 1), np.float32)
    G = np.zeros((NCORES, 64, NG), np.float32)
    for c in range(NCORES):
        bt = batch[c * NPC : (c + 1) * NPC] - g0[c]
        br = np.full(NLOC, -1.0, np.float64)
        br[:NPC] = bt
        batchrel[c] = br.reshape(NBLK, 128, 1).astype(nbf)
        wn = np.zeros(NLOC, np.float32)
        wn[:NPC] = 1.0 / np.maximum(cnt[batch[c * NPC : (c + 1) * NPC]], 1.0)
        wnode[c] = wn.reshape(NBLK, 128, 1)
        for i in range(64):
            g = g0[c] + i
            if g < NG:
                G[c, i, g] = 1.0

    iota = np.tile(np.arange(128, dtype=nbf)[None, :], (128, 1))
    iotac = np.arange(128, dtype=np.float32).reshape(128, 1)
    # dstrel in row layout [NBLK, NT, 128] f32 (slot s of tile t -> col s%128)
    dstrel_row = np.ascontiguousarray(dstrel.astype(np.float32).transpose(0, 1, 3, 2))  # [NC, NBLK, NT, 128]

    # xT per core: [KPAD, NLOC] f32
    xT = np.zeros((NCORES, KPAD, NLOC), np.float32)
    for c in range(NCORES):
        xT[c, :IN_C, :NPC] = x[c * NPC : (c + 1) * NPC].T

    g = lambda k: np.asarray(inputs[k], np.float32)
    W = {}
    W["syn_W"] = np.zeros((KPAD, HID), np.float32); W["syn_W"][:IN_C] = g("syn_W")
    W["ant_W"] = np.zeros((KPAD, HID), np.float32); W["ant_W"][:IN_C] = g("ant_W")
    W["syn_b"] = g("syn_b").reshape(HID, 1)
    W["ant_b"] = g("ant_b").reshape(HID, 1)
    W["fus_W"] = g("fus_W")  # [128, 128]
    W["fus_b"] = g("fus_b").reshape(128, 1)
    sc = 1.0 / np.sqrt(np.float32(HID))
    W["c1_Wq"] = g("c1_Wq") * sc
    W["c1_bq"] = g("c1_bq") * sc
    W["c1_Wkv"] = np.concatenate([g("c1_Wk"), g("c1_Wv")], axis=1)  # [128, 256]
    W["c1_Ws"] = g("c1_Ws")
    W["c2_Wq"] = g("c2_Wq") * sc
    W["c2_bq"] = g("c2_bq") * sc
    W["c2_Wkv"] = np.concatenate([g("c2_Wk"), g("c2_Wv")], axis=1)  # [128, 128]
    W["c2_Ws"] = g("c2_Ws")
    W["l1_W"] = g("l1_W")
    W["l1_b"] = g("l1_b").reshape(HID, 1)
    W["l2_W"] = g("l2_W")
    W["l2_b"] = g("l2_b").reshape(OUT_C, 1)

    # bias rows for K=1 matmul accumulation
    kv1_brow = np.concatenate([g("c1_bk"), g("c1_bv")]).reshape(1, 256).astype(np.float32)
    q1_brow = W["c1_bq"].reshape(1, 128).astype(np.float32)
    s1_brow = g("c1_bs").reshape(1, 128).astype(np.float32)
    kv2_brow = np.concatenate([g("c2_bk"), g("c2_bv")]).reshape(1, 128).astype(np.float32)
    q2_brow = W["c2_bq"].reshape(1, HID).astype(np.float32)
    s2_brow = g("c2_bs").reshape(1, HID).astype(np.float32)

    base_map = {
        "iota": iota,
        "syn_W": W["syn_W"], "ant_W": W["ant_W"],
        "syn_b": W["syn_b"], "ant_b": W["ant_b"],
        "fus_W": W["fus_W"], "fus_b": W["fus_b"],
        "c1_Wq": W["c1_Wq"], "c1_Wkv": W["c1_Wkv"], "c1_Ws": W["c1_Ws"],
        "c2_Wq": W["c2_Wq"], "c2_Wkv": W["c2_Wkv"], "c2_Ws": W["c2_Ws"],
        "l1_W": W["l1_W"], "l1_b": W["l1_b"], "l2_W": W["l2_W"], "l2_b": W["l2_b"],
        "kv1_brow": kv1_brow, "q1_brow": q1_brow, "s1_brow": s1_brow,
        "kv2_brow": kv2_brow, "q2_brow": q2_brow, "s2_brow": s2_brow,
        "iotac": iotac,
    }
    in_maps = []
    for c in range(NCORES):
        m = dict(base_map)
        m.update({
            "xT": xT[c],
            "kv_ilo": kv_ilo[c], "kv_ihi": kv_ihi[c],
            "q_ilo": q_ilo[c], "q_ihi": q_ihi[c],
            "dstrel": dstrel[c], "dstrel_row": dstrel_row[c],
            "batchrel": batchrel[c], "wnode": wnode[c],
            "G": G[c],
        })
        in_maps.append(m)
    caps = dict(CAPL=CAPL, CAPH=CAPH, NTL=NTL, NTH=NTH, NT=NT)
    return in_maps, caps


def _build(caps, phases=99):
    CAPL, CAPH, NTL, NTH, NT = caps["CAPL"], caps["CAPH"], caps["NTL"], caps["NTH"], caps["NT"]
    nc = bacc.Bacc("TRN2", target_bir_lowering=False, num_devices=NCORES, num_swdge_queues=4)
    P = {}

    def inp(name, shape, dt):
        P[name] = nc.declare_dram_parameter(name, list(shape), dt, isOutput=False)
        return P[name]

    inp("xT", [KPAD, NLOC], f32)
    inp("iota", [128, 128], bf16)
    inp("syn_W", [KPAD, HID], f32); inp("ant_W", [KPAD, HID], f32)
    inp("syn_b", [HID, 1], f32); inp("ant_b", [HID, 1], f32)
    inp("fus_W", [128, 128], f32); inp("fus_b", [128, 1], f32)
    inp("c1_Wq", [128, 128], f32); inp("c1_Wkv", [128, 256], f32); inp("c1_Ws", [128, 128], f32)
    inp("c2_Wq", [128, HID], f32); inp("c2_Wkv", [128, 128], f32); inp("c2_Ws", [128, HID], f32)
    inp("l1_W", [HID, HID], f32); inp("l1_b", [HID, 1], f32)
    inp("l2_W", [HID, OUT_C], f32); inp("l2_b", [OUT_C, 1], f32)
    inp("kv1_brow", [1, 256], f32); inp("q1_brow", [1, 128], f32); inp("s1_brow", [1, 128], f32)
    inp("kv2_brow", [1, 128], f32); inp("q2_brow", [1, HID], f32); inp("s2_brow", [1, HID], f32)
    inp("iotac", [128, 1], f32)
    inp("kv_ilo", [NBLK, 128, CAPL // 16], i16); inp("kv_ihi", [NBLK, 128, CAPH // 16], i16)
    inp("q_ilo", [NBLK, 128, CAPL // 16], i16); inp("q_ihi", [NBLK, 128, CAPH // 16], i16)
    inp("dstrel", [NBLK, 128, NT], bf16)
    inp("dstrel_row", [NBLK, NT, 128], f32)
    inp("batchrel", [NBLK, 128, 1], bf16); inp("wnode", [NBLK, 128, 1], f32)
    inp("G", [64, NG], f32)
    out_t = nc.declare_dram_parameter("out", [OUT_C, NG], f32, isOutput=True)

    xfT_loc = nc.dram_tensor("xfT_loc", [128, NLOC], f32)
    h1T_loc = nc.dram_tensor("h1T_loc", [128, NLOC], f32)
    kv1_tab = nc.dram_tensor("kv1_tab", [NPAD, 256], bf16)
    q1_tab = nc.dram_tensor("q1_tab", [NLOC, 128], bf16)
    skip1_tab = nc.dram_tensor("skip1_tab", [NLOC, 128], f32)
    kv2_tab = nc.dram_tensor("kv2_tab", [NPAD, 128], f32)
    q2_tab = nc.dram_tensor("q2_tab", [NLOC, HID], f32)
    skip2_tab = nc.dram_tensor("skip2_tab", [NLOC, HID], f32)

    Exp = mybir.ActivationFunctionType.Exp
    Copy = mybir.ActivationFunctionType.Copy
    Relu = mybir.ActivationFunctionType.Relu
    EQ = mybir.AluOpType.is_equal
    MUL = mybir.AluOpType.mult
    ADD = mybir.AluOpType.add
    AX = mybir.AxisListType.X

    with tile.TileContext(nc) as tc:
        with (
            tc.tile_pool(name="cst", bufs=1) as cst,
            tc.tile_pool(name="wpool", bufs=1) as wp,
            tc.tile_pool(name="stage", bufs=3) as st,
            tc.tile_pool(name="gath", bufs=2) as gp,
            tc.tile_pool(name="att", bufs=2) as at,
            tc.tile_pool(name="ps_stage", bufs=2, space="PSUM") as ps_st,
            tc.tile_pool(name="ps_att", bufs=3, space="PSUM") as ps_at,
            tc.tile_pool(name="ps_fix", bufs=1, space="PSUM") as ps_fix,
            tc.tile_pool(name="dram", bufs=1, space="DRAM") as dram,
        ):
            iota = cst.tile([128, 128], bf16)
            nc.sync.dma_start(out=iota[:], in_=P["iota"][:])
            iotac = cst.tile([128, 1], f32)
            nc.sync.dma_start(out=iotac[:], in_=P["iotac"][:])
            ones1 = cst.tile([1, 128], f32)
            nc.vector.memset(ones1[:], 1.0)
            brows = {}
            for bn, bw in (("kv1_brow", 256), ("q1_brow", 128), ("s1_brow", 128),
                           ("kv2_brow", 128), ("q2_brow", HID), ("s2_brow", HID)):
                brows[bn] = cst.tile([1, bw], f32, tag=bn, name=bn)
                nc.sync.dma_start(out=brows[bn][:], in_=P[bn][:])
            ident = cst.tile([128, 128], f32)
            make_identity(nc, ident[:])

            def wtile(name, shape, dt=f32):
                t = wp.tile(list(shape), dt, tag=name)
                nc.sync.dma_start(out=t[:], in_=P[name][:])
                return t

            syn_W = wp.tile([128, 3, HID], f32, tag="syn_W")
            nc.sync.dma_start(out=syn_W[:], in_=P["syn_W"][:].rearrange("(s p) h -> p s h", p=128))
            ant_W = wp.tile([128, 3, HID], f32, tag="ant_W")
            nc.sync.dma_start(out=ant_W[:], in_=P["ant_W"][:].rearrange("(s p) h -> p s h", p=128))
            syn_b = wtile("syn_b", [HID, 1]); ant_b = wtile("ant_b", [HID, 1])
            fus_W = wtile("fus_W", [128, 128]); fus_b = wtile("fus_b", [128, 1])
            c1_Wq = wtile("c1_Wq", [128, 128]); c1_Wkv = wtile("c1_Wkv", [128, 256])
            c1_Ws = wtile("c1_Ws", [128, 128])
            c2_Wq = wtile("c2_Wq", [128, HID]); c2_Wkv = wtile("c2_Wkv", [128, 128])
            c2_Ws = wtile("c2_Ws", [128, HID])
            l1_W = wtile("l1_W", [HID, HID]); l1_b = wtile("l1_b", [HID, 1])
            l2_W = wtile("l2_W", [HID, OUT_C]); l2_b = wtile("l2_b", [OUT_C, 1])
            Gt = wtile("G", [64, NG])

            # ---- stage 1: x -> x_fusedT  [128, NLOC] ----
            NJ = 512
            for j in range(NLOC // NJ + (1 if NLOC % NJ else 0)):
                j0 = j * NJ
                nj = min(NJ, NLOC - j0)
                xT_sb = st.tile([128, 3, nj], f32, tag="xT_sb")
                nc.sync.dma_start(
                    out=xT_sb[:],
                    in_=P["xT"][:, j0 : j0 + nj].rearrange("(s p) n -> p s n", p=128),
                )
                comb = st.tile([128, nj], f32, tag="comb")
                for half, (Wt, bt) in enumerate(((syn_W, syn_b), (ant_W, ant_b))):
                    acc = ps_st.tile([HID, nj], f32, tag="ps", space="PSUM")
                    for s in range(3):
                        nc.tensor.matmul(
                            out=acc[:], lhsT=Wt[:, s, :],
                            rhs=xT_sb[:, s, :], start=(s == 0), stop=(s == 2))
                    nc.scalar.activation(
                        out=comb[half * HID : (half + 1) * HID, :], in_=acc[:],
                        func=Relu, bias=bt[:])
                xf_ps = ps_st.tile([128, nj], f32, tag="ps", space="PSUM")
                nc.tensor.matmul(out=xf_ps[:], lhsT=fus_W[:], rhs=comb[:],
                                 start=True, stop=True)
                xf_sb = st.tile([128, nj], f32, tag="xf_sb")
                nc.vector.tensor_scalar_add(out=xf_sb[:], in0=xf_ps[:], scalar1=fus_b[:])
                nc.sync.dma_start(out=xfT_loc[:, j0 : j0 + nj], in_=xf_sb[:])

            # ---- AllGather x_fusedT ----
            ag1_in = dram.tile([128, NLOC], f32)
            ag1_out = dram.tile([NCORES, 128, NLOC], f32)
            nc.gpsimd.dma_start(out=ag1_in[:], in_=xfT_loc[:])
            nc.gpsimd.collective_compute(
                "AllGather", mybir.AluOpType.bypass,
                replica_groups=[list(range(NCORES))],
                ins=[ag1_in.opt()], outs=[ag1_out.opt()])

            # ---- build q1/skip1 (local) ----
            def build_tab(lh, Wt, brow, ncols, tab, rows, out_dt, eng):
                pps = ps_st.tile([128, ncols], f32, tag="ps", space="PSUM")
                nc.tensor.matmul(out=pps[:], lhsT=lh[:], rhs=Wt[:], start=True, stop=False)
                nc.tensor.matmul(out=pps[:], lhsT=ones1[:], rhs=brow[:], start=False, stop=True)
                sb = st.tile([128, ncols], out_dt, tag="tb_" + eng)
                if eng == "act":
                    nc.scalar.activation(out=sb[:], in_=pps[:], func=Copy)
                else:
                    nc.vector.tensor_copy(out=sb[:], in_=pps[:])
                nc.sync.dma_start(out=tab[rows, :], in_=sb[:])

            for t in range(NBLK if phases >= 2 else 0):
                lh = st.tile([128, 128], f32, tag="lh_loc1")
                nc.sync.dma_start(out=lh[:], in_=xfT_loc[:, t * 128 : (t + 1) * 128])
                rows = slice(t * 128, (t + 1) * 128)
                build_tab(lh, c1_Wq, brows["q1_brow"], 128, q1_tab, rows, bf16, "act")
                build_tab(lh, c1_Ws, brows["s1_brow"], 128, skip1_tab, rows, f32, "dve")

            # ---- build kv1 (all ranks) ----
            for t in range(NCORES * NBLK if phases >= 2 else 0):
                r, tl = t // NBLK, t % NBLK
                lh = st.tile([128, 128], f32, tag="lh_kv1")
                nc.sync.dma_start(out=lh[:], in_=ag1_out[r, :, tl * 128 : (tl + 1) * 128])
                build_tab(lh, c1_Wkv, brows["kv1_brow"], 256, kv1_tab,
                          slice(t * 128, (t + 1) * 128), bf16,
                          "act" if t % 2 == 0 else "dve")

            # ---- conv1 ----
            def conv_block(b, kv_tab, q_tab, skip_tab, kv_elem, q_elem, heads,
                           kv_dt, q_dt, tagp, relu_out):
                """Returns SBUF tile [128, heads*64] f32 (attn+skip[, relu])."""
                import os as _os
                parts = int(_os.environ.get("KCPARTS", "9"))
                dh = 64
                fdim = heads * dh
                kvj = gp.tile([128, NT, kv_elem], kv_dt, tag=tagp + "kvj")
                if b <= 1:  # scrub first-use garbage in both pool bufs
                    nc.vector.memset(kvj[:], 0.0)
                qj = gp.tile([128, NT, q_elem], q_dt, tag=tagp + "qj")
                if b <= 1:
                    nc.vector.memset(qj[:], 0.0)
                ilo = gp.tile([128, CAPL // 16], i16, tag=tagp + "ilo")
                ihi = gp.tile([128, CAPH // 16], i16, tag=tagp + "ihi")
                qlo = gp.tile([128, CAPL // 16], i16, tag=tagp + "qlo")
                qhi = gp.tile([128, CAPH // 16], i16, tag=tagp + "qhi")
                dstl = gp.tile([128, NT], bf16, tag=tagp + "dstl")
                nc.sync.dma_start(out=ilo[:], in_=P["kv_ilo"][b])
                nc.sync.dma_start(out=ihi[:], in_=P["kv_ihi"][b])
                nc.sync.dma_start(out=qlo[:], in_=P["q_ilo"][b])
                nc.sync.dma_start(out=qhi[:], in_=P["q_ihi"][b])
                nc.sync.dma_start(out=dstl[:], in_=P["dstrel"][b])

                qcyc = [0]
                def chunked_gather(out_tile, tile0, in_ap, idxs, cap, elem):
                    s = 0
                    while s < cap:
                        n = min(1024, cap - s)
                        nc.gpsimd.dma_gather(
                            out_ap=out_tile[:, tile0 + s // 128 : tile0 + (s + n) // 128, :],
                            in_ap=in_ap, idxs_ap=idxs[:, s // 16 : (s + n) // 16],
                            num_idxs=n, num_idxs_reg=n, elem_size=elem,
                            queue_num=qcyc[0] % int(__import__("os").environ.get("KNQ", "4")))
                        qcyc[0] += 1
                        s += n

                chunked_gather(kvj, 0, kv_tab[0:SPLIT, :], ilo, CAPL, kv_elem)
                chunked_gather(kvj, NTL, kv_tab[SPLIT:NPAD, :], ihi, CAPH, kv_elem)
                chunked_gather(qj, 0, q_tab[:], qlo, CAPL, q_elem)
                chunked_gather(qj, NTL, q_tab[:], qhi, CAPH, q_elem)



                if parts < 2:
                    hout = at.tile([128, fdim], f32, tag=tagp + "hout")
                    nc.vector.tensor_copy(out=hout[:, 0:kv_elem//2], in_=kvj[:, 0, 0:kv_elem//2])
                    return hout
                # alpha = per-edge q.k, all tiles merged
                prod = at.tile([128, NT, fdim], f32, tag=tagp + "prod")
                nc.vector.tensor_tensor(out=prod[:], in0=qj[:], in1=kvj[:, :, 0:fdim], op=MUL)
                alpha = at.tile([128, NT * heads], f32, tag=tagp + "alpha")
                nc.vector.reduce_sum(
                    out=alpha[:],
                    in_=prod[:].rearrange("p t (h d) -> p (t h) d", d=dh), axis=AX)
                ea = at.tile([128, NT * heads], f32, tag=tagp + "ea")
                nc.scalar.activation(out=ea[:], in_=alpha[:], func=Exp)

                if parts < 3:
                    hout = at.tile([128, fdim], f32, tag=tagp + "hout")
                    nc.vector.tensor_copy(out=hout[:, 0:heads], in_=ea[:, 0:heads])
                    return hout
                wv = at.tile([128, NT, fdim + heads], bf16, tag=tagp + "wv")
                for t in range(NT):
                    for h in range(heads):
                        nc.scalar.activation(
                            out=wv[:, t, h * dh : (h + 1) * dh],
                            in_=kvj[:, t, fdim + h * dh : fdim + (h + 1) * dh],
                            func=Copy, scale=ea[:, t * heads + h : t * heads + h + 1])
                nc.vector.tensor_copy(
                    out=wv[:, :, fdim : fdim + heads],
                    in_=ea[:].rearrange("p (t h) -> p t h", h=heads))

                if parts < 4:
                    hout = at.tile([128, fdim], f32, tag=tagp + "hout")
                    nc.vector.tensor_copy(out=hout[:], in_=wv[:, 0, 0:fdim])
                    return hout
                numden = ps_at.tile([128, fdim + heads], f32, tag="numden", space="PSUM")
                for t in range(NT):
                    stt = at.tile([128, 128], bf16, tag=tagp + "st")
                    nc.vector.tensor_tensor(
                        out=stt[:], in0=dstl[:, t : t + 1].to_broadcast([128, 128]),
                        in1=iota[:], op=EQ)
                    nc.tensor.matmul(out=numden[:], lhsT=stt[:], rhs=wv[:, t, :],
                                     start=(t == 0), stop=(t == NT - 1))

                dinv = at.tile([128, heads], f32, tag=tagp + "dinv")
                nc.vector.tensor_scalar_add(out=dinv[:], in0=numden[:, fdim:], scalar1=1e-16)
                nc.vector.reciprocal(out=dinv[:], in_=dinv[:])
                res = at.tile([128, fdim], f32, tag=tagp + "res")
                for h in range(heads):
                    nc.scalar.activation(
                        out=res[:, h * dh : (h + 1) * dh],
                        in_=numden[:, h * dh : (h + 1) * dh],
                        func=Copy, scale=dinv[:, h : h + 1])
                if parts < 5:
                    hout = at.tile([128, fdim], f32, tag=tagp + "hout")
                    nc.vector.tensor_copy(out=hout[:], in_=res[:])
                    return hout
                skip = at.tile([128, fdim], f32, tag=tagp + "skip")
                nc.sync.dma_start(out=skip[:], in_=skip_tab[b * 128 : (b + 1) * 128, :])
                hout = at.tile([128, fdim], f32, tag=tagp + "hout")
                nc.vector.tensor_tensor(out=hout[:], in0=res[:], in1=skip[:], op=ADD)
                return hout

            import os as _os
            _c1blks = min(NBLK, int(_os.environ.get("KC1BLKS", str(NBLK))))
            for b in range(_c1blks if phases >= 3 else 0):
                hout = conv_block(b, kv1_tab, q1_tab, skip1_tab,
                                  256, 128, 2, bf16, bf16, "c1", True)
                # transpose + relu -> h1T
                h1T_ps = ps_fix.tile([128, 128], f32, tag="fix2", space="PSUM")
                nc.tensor.transpose(out=h1T_ps[:], in_=hout[:], identity=ident[:])
                h1T_sb = at.tile([128, 128], f32, tag="h1T_sb")
                nc.scalar.activation(out=h1T_sb[:], in_=h1T_ps[:], func=Relu)
                nc.sync.dma_start(out=h1T_loc[:, b * 128 : (b + 1) * 128], in_=h1T_sb[:])

            # ---- AllGather h1T ----
            ag2_in = dram.tile([128, NLOC], f32)
            ag2_out = dram.tile([NCORES, 128, NLOC], f32)
            nc.gpsimd.dma_start(out=ag2_in[:], in_=h1T_loc[:])
            nc.gpsimd.collective_compute(
                "AllGather", mybir.AluOpType.bypass,
                replica_groups=[list(range(NCORES))],
                ins=[ag2_in.opt()], outs=[ag2_out.opt()])

            # ---- build q2/skip2 (local) + kv2 (all ranks) ----
            for t in range(NBLK if phases >= 4 else 0):
                lh = st.tile([128, 128], f32, tag="lh_loc2")
                nc.sync.dma_start(out=lh[:], in_=h1T_loc[:, t * 128 : (t + 1) * 128])
                rows = slice(t * 128, (t + 1) * 128)
                build_tab(lh, c2_Wq, brows["q2_brow"], HID, q2_tab, rows, f32, "act")
                build_tab(lh, c2_Ws, brows["s2_brow"], HID, skip2_tab, rows, f32, "dve")
            for t in range(NCORES * NBLK if phases >= 4 else 0):
                r, tl = t // NBLK, t % NBLK
                lh = st.tile([128, 128], f32, tag="lh_kv2")
                nc.sync.dma_start(out=lh[:], in_=ag2_out[r, :, tl * 128 : (tl + 1) * 128])
                build_tab(lh, c2_Wkv, brows["kv2_brow"], 128, kv2_tab,
                          slice(t * 128, (t + 1) * 128), f32,
                          "act" if t % 2 == 0 else "dve")

            # ---- conv2 + pooling ----
            poolS = ps_fix.tile([64, HID], f32, tag="poolS", space="PSUM")
            for b in range(NBLK if phases >= 5 else 0):
                hout = conv_block(b, kv2_tab, q2_tab, skip2_tab,
                                  128, 64, 1, f32, f32, "c2", False)
                wcol = at.tile([128, 1], f32, tag="wcol")
                nc.sync.dma_start(out=wcol[:], in_=P["wnode"][b])
                brel = at.tile([128, 1], bf16, tag="brel")
                nc.sync.dma_start(out=brel[:], in_=P["batchrel"][b])
                h2w = at.tile([128, HID], f32, tag="h2w")
                nc.scalar.activation(out=h2w[:], in_=hout[:], func=Copy, scale=wcol[:])
                pblk = at.tile([128, 64], f32, tag="pblk")
                nc.vector.tensor_tensor(
                    out=pblk[:], in0=brel[:].to_broadcast([128, 64]),
                    in1=iota[:, 0:64], op=EQ)
                nc.tensor.matmul(out=poolS[:], lhsT=pblk[:], rhs=h2w[:],
                                 start=(b == 0), stop=(b == NBLK - 1))

            # poolS [64 gslot, 64 f] -> pooledT contrib [64 f, NG]
            poolS_sb = st.tile([64, HID], f32, tag="poolS_sb")
            if phases >= 5:
                nc.vector.tensor_copy(out=poolS_sb[:], in_=poolS[:])
            else:
                nc.vector.memset(poolS_sb[:], 0.0)
            pooledT_ps = ps_fix.tile([HID, NG], f32, tag="fix2", space="PSUM")
            nc.tensor.matmul(out=pooledT_ps[:], lhsT=poolS_sb[:], rhs=Gt[:],
                             start=True, stop=True)
            pooledT_sb = st.tile([HID, NG], f32, tag="pooledT_sb")
            nc.vector.tensor_copy(out=pooledT_sb[:], in_=pooledT_ps[:])

            ar_in = dram.tile([HID, NG], f32)
            ar_out = dram.tile([HID, NG], f32)
            nc.gpsimd.dma_start(out=ar_in[:], in_=pooledT_sb[:])
            nc.gpsimd.collective_compute(
                "AllReduce", mybir.AluOpType.add,
                replica_groups=[list(range(NCORES))],
                ins=[ar_in.opt()], outs=[ar_out.opt()])
            pooled_all = st.tile([HID, NG], f32, tag="pooled_all")
            nc.sync.dma_start(out=pooled_all[:], in_=ar_out[:])

            z1_ps = ps_fix.tile([HID, NG], f32, tag="fix2", space="PSUM")
            nc.tensor.matmul(out=z1_ps[:], lhsT=l1_W[:], rhs=pooled_all[:],
                             start=True, stop=True)
            z1_sb = st.tile([HID, NG], f32, tag="z1_sb")
            nc.scalar.activation(out=z1_sb[:], in_=z1_ps[:], func=Relu, bias=l1_b[:])
            z2_ps = ps_fix.tile([OUT_C, NG], f32, tag="fix2", space="PSUM")
            nc.tensor.matmul(out=z2_ps[:], lhsT=l2_W[:], rhs=z1_sb[:],
                             start=True, stop=True)
            z2_sb = st.tile([OUT_C, NG], f32, tag="z2_sb")
            nc.vector.tensor_scalar_add(out=z2_sb[:], in0=z2_ps[:], scalar1=l2_b[:])
            nc.sync.dma_start(out=out_t[:], in_=z2_sb[:])

    nc.compile()
    return nc


_CACHE = {}


def run(inputs, trace=False):
    in_maps, caps = _prep(inputs)
    key = (caps["CAPL"], caps["CAPH"])
    import os
    phases = int(os.environ.get("KPHASES", "99"))
    key = (key, phases)
    if key not in _CACHE:
        _CACHE[key] = _build(caps, phases)
    nc = _CACHE[key]
    res = run_bass_kernel_spmd(nc, in_maps, core_ids=list(range(NCORES)), trace=trace)
    out = np.asarray(res.results[0]["out"], np.float32)  # [2, NG]
    return np.ascontiguousarray(out.T), res.exec_time_ns


def kernel(**inputs):
    out, _ = run(inputs)
    return out
